# revision 5
# baseline (speedup 1.0000x reference)
"""AliasFreeActivation Trainium2 kernel (v22).

out = crop10(down2(leaky_relu(up4(x + bias)) * sqrt2))   [4,256,236,236]

Per (batch,channel) image (1024 images, 128 per core), with A the up4
matrix [128,512] and D the down2 matrix [512,256] (both banded):

  s1  v1[w,ho]  = sum_h xb[h,w] A[h,ho]              1 MM N=512
  s2  Y[ho,j]   = sum_w v1[w,ho] (sqrt2*A)[w,15+j]   4 MM N=483
      (fine-grid cols wo outside [15,498) never reach the cropped
       output, so they are never produced: N=483 not 512)
  act L = prelu(Y, 0.2)   one fused ScalarE pass per PSUM pair
          (= sqrt2*leaky_relu(up4(xb)); sqrt2 folded into s2's matrix;
           ACT Prelu honors alpha — Lrelu does NOT, its slope is 0.01)
  s3  z[wo,hd]  = sum_ho L[ho,wo] D[ho,hd]          16 MM banded N<=70
      (cropped-wo m-chunks {0,128,256,355}; the last chunk overlaps the
       third by 29 rows so every MM keeps a full-128-partition output —
       the doubled rows are zeroed in the s4 D-blocks)
  s4  oT[wd,hd] = sum_wo D[wo,wd] z[wo,hd]           5 MM N=236
      (constant D stationary -> no per-image weight loads scale with the
       data; output lands transposed and is fixed on the host)

ScalarE is the wall: every fine-grid element must cross PSUM->SBUF
through ACT or DVE at 1 elem/cycle/lane (trn2 matmul output is fp32-only,
so no 16-bit 2x DVE modes), and the trace shows ACT 100% back-to-back.
Budget per image: ACT = prelu 2x(966+~240)/1.2 + 162-col share of the
second output group's copy; DVE = V1/z/out casts + the other 74 cols;
both land ~2.34us.  Startup: the prelu table set is preloaded via a
dummy activation on a memset scratch during the const DMA, and the
A/A2 half of the const matrix is DMA'd separately so s1/s2 unblock
before the D-blocks arrive.  I/O is fp16 both ways, one input and one
output DMA per image.  s4 accumulates both 128-row output groups in ONE
PSUM bank (start=True only on the bank's first MM).

Partial-partition matmul outputs (M<128) accumulate incorrectly on this
stack — all matmuls here write full-128-partition outputs.
"""
import numpy as np

UP, DOWN, MARGIN, NEG_SLOPE = 4, 2, 10, 0.2
SQRT2 = 1.4142135623730951
H = W = 128
OUT = 236
NCORES = 8
NIMG = 128
WO0, WON = 15, 483    # fine-grid wo crop: orig wo = cropped + 15
MC = [0, 128, 256, 355]          # cropped-wo m-chunk starts (s3 lhsT)
OFSPLIT = 95                     # cols of OF group-1 evacuated on ACT

# down-matrix window per 128-row K-chunk: D[s,o] nonzero for o in [64k-3, 64k+66]
DWIN = [(0, 67), (61, 131), (125, 195), (189, 256)]
# s4 blocks (chunk k, out-group g) with g0 = wd 10..137, g1 = wd 138..265
S4MM = [(0, 0), (1, 0), (2, 0), (2, 1), (3, 1)]

CM_A = 0
CM_A2 = 512           # 483 cols (sqrt2*A, wo-cropped)
CM_DW = 996           # 4 windows, 70 cols apart (4-elem aligned)
CM_DH = 1280          # 5 dense [128,128] blocks (4-elem aligned)
CM_COLS = 1920
CM_SPLIT = 996        # DMA chunk 1 = A|A2 (unblocks s1/s2 early)
VERSION = 79          # bump on every kernel change: cache-key nonce

_cache = {}


def _build_nc(nimg=NIMG):
    import concourse.bacc as bacc
    import concourse.bass as bass
    import concourse.tile as tile
    from concourse import mybir

    F32 = mybir.dt.float32
    F16 = mybir.dt.float16
    AF = mybir.ActivationFunctionType
    ALU = mybir.AluOpType

    nc = bacc.Bacc("TRN2", target_bir_lowering=False)
    x_d = nc.dram_tensor("x", [nimg, H, W], F16, kind="ExternalInput")
    c_d = nc.dram_tensor("cm", [128, CM_COLS], F16, kind="ExternalInput")
    nc.dram_tensor("nonce", [1, VERSION], F16, kind="ExternalInput")
    o_d = nc.dram_tensor("out", [nimg, 2, 128, OUT], F16, kind="ExternalOutput")

    with tile.TileContext(nc) as tc:
        with (
            tc.tile_pool(name="const", bufs=1) as const,
            tc.tile_pool(name="xin", bufs=10) as xin,
            tc.tile_pool(name="v1p", bufs=3) as v1p,
            tc.tile_pool(name="yp", bufs=3) as yp,
            tc.tile_pool(name="zp", bufs=3) as zp,
            tc.tile_pool(name="ofp", bufs=6) as ofp,
            tc.tile_pool(name="p1", bufs=1, space="PSUM") as p1p,
            tc.tile_pool(name="p2", bufs=2, space="PSUM") as p2p,
            tc.tile_pool(name="p34", bufs=1, space="PSUM") as p34p,
            tc.tile_pool(name="pt", bufs=1, space="PSUM") as ptp,
        ):
            # warm the ACT table set (prelu) on a memset scratch so the
            # ~2.7us ACT_TABLE_LOAD overlaps the const DMA
            warm = const.tile([1, 2], F16)
            nc.vector.memset(warm[0:1, 0:1], 0.0)
            nc.scalar.activation(out=warm[0:1, 1:2], in_=warm[0:1, 0:1],
                                 func=AF.Prelu, bias=0.0, scale=1.0,
                                 alpha=NEG_SLOPE)

            cm = const.tile([128, CM_COLS], F16)
            nc.sync.dma_start(out=cm[:, 0:CM_SPLIT], in_=c_d[:, 0:CM_SPLIT])
            nc.sync.dma_start(out=cm[:, CM_SPLIT:CM_COLS],
                              in_=c_d[:, CM_SPLIT:CM_COLS])
            A_sb = cm[:, CM_A:CM_A + 512]
            A2_sb = cm[:, CM_A2:CM_A2 + WON]

            def D_sb(k):
                o0, o1 = DWIN[k]
                return cm[:, CM_DW + 70 * k: CM_DW + 70 * k + (o1 - o0)]

            def Dh_sb(j):
                return cm[:, CM_DH + 128 * j: CM_DH + 128 * (j + 1)]

            # warm PE's clock on the const DMA lane
            pwarm = p2p.tile([128, 2, 512], F32, name="p2")
            nc.tensor.matmul(out=pwarm[:32, 0, :256], lhsT=cm[:, :32],
                             rhs=cm[:, :256], start=True, stop=True)

            def s1_mm(i):
                # s1: up vertical (bias folds in during evacuation)
                X = xin.tile([128, W], F16)
                nc.sync.dma_start(out=X, in_=x_d[i])
                P1 = p1p.tile([128, 512], F32)
                nc.tensor.matmul(out=P1, lhsT=X, rhs=A_sb,
                                 start=True, stop=True)
                return P1

            def s1_evac(i, P1):
                V1 = v1p.tile([128, 512], F16)
                nc.vector.tensor_copy(out=V1, in_=P1)
                return V1

            def s4_mm(ip, Z):
                # s4: down horizontal with D stationary -> transposed out
                PT = ptp.tile([128, 2, OUT], F32)
                for j, (k, g) in enumerate(S4MM):
                    nc.tensor.matmul(out=PT[:, g, :], lhsT=Dh_sb(j),
                                     rhs=Z[:, k, :],
                                     start=(j == 0), stop=(j == len(S4MM) - 1))
                return (ip, PT)

            def of_evac(ip, PT):
                # deferred one iteration: deps are long done, so these
                # never block the engine FIFOs
                OF = ofp.tile([128, 2, OUT], F16)
                nc.vector.tensor_copy(out=OF[:, 0, :], in_=PT[:, 0, :])
                # split g1 along the free dim (partition base must stay 0
                # for PSUM reads) to balance ACT vs DVE busy time
                nc.scalar.copy(out=OF[0:OUT - 128, 1, 0:OFSPLIT],
                               in_=PT[0:OUT - 128, 1, 0:OFSPLIT])
                nc.vector.tensor_copy(out=OF[0:OUT - 128, 1, OFSPLIT:OUT],
                                      in_=PT[0:OUT - 128, 1, OFSPLIT:OUT])
                nc.sync.dma_start(
                    out=bass.AP(tensor=o_d[:].tensor,
                                offset=ip * 2 * 128 * OUT,
                                ap=[[OUT, 128], [128 * OUT, 2], [1, OUT]]),
                    in_=OF)

            def s2_act(V1):
                # s2 + fused leaky-relu evacuation (fine grid, wo-cropped)
                Y = yp.tile([128, 4, WON], F16)
                for pr in range(2):
                    P2 = p2p.tile([128, 2, 512], F32, name="p2")
                    for h in range(2):
                        m = 2 * pr + h
                        nc.tensor.matmul(out=P2[:, h, 0:WON],
                                         lhsT=V1[:, 128 * m:128 * (m + 1)],
                                         rhs=A2_sb, start=True, stop=True)
                    nc.scalar.activation(out=Y[:, 2 * pr:2 * pr + 2, :],
                                         in_=P2[:, :, 0:WON], func=AF.Prelu,
                                         bias=0.0, scale=1.0, alpha=NEG_SLOPE)
                return Y

            def s3_z(Y):
                # s3: down vertical (banded), all four wo-chunks in one
                # 2-bank PSUM tile, single evacuation
                P34 = p34p.tile([128, 4, 256], F32)
                for m in range(4):
                    for k in range(4):
                        o0, o1 = DWIN[k]
                        nc.tensor.matmul(
                            out=P34[:, m, o0:o1],
                            lhsT=Y[:, k, MC[m]:MC[m] + 128],
                            rhs=D_sb(k), start=(k == 0), stop=(k == 3))
                Z = zp.tile([128, 4, OUT], F16)
                nc.vector.tensor_copy(out=Z,
                                      in_=P34[:, :, MARGIN:MARGIN + OUT])
                return Z

            # software pipeline: s3/s4 of image i-1 are emitted during
            # iteration i, and of_evac lags two images, so every engine
            # reaches each instruction with its producers long finished
            # (the s4->PT->copy chain is ~2.4us; ACT's own work per image
            # is ~2.0us, so a lag-1 copy stalls ACT)
            Yprev = None
            PTlag = None
            for i in range(nimg):
                V1 = s1_evac(i, s1_mm(i))
                Y = s2_act(V1)
                if Yprev is not None:
                    Z = s3_z(Yprev)
                    if PTlag is not None:
                        of_evac(*PTlag)
                    PTlag = s4_mm(i - 1, Z)
                Yprev = Y

            Z = s3_z(Yprev)
            of_evac(*PTlag)
            PTlag = s4_mm(nimg - 1, Z)
            of_evac(*PTlag)

    nc.finalize()
    return nc


def _filter_matrices(up_filter, down_filter):
    fu = np.asarray(up_filter, dtype=np.float64)
    fd = np.asarray(down_filter, dtype=np.float64)
    i = np.arange(128)[:, None]
    o = np.arange(512)[None, :]
    t = 10 + o - 4 * i
    A = np.where((t >= 0) & (t < 24), fu[np.clip(t, 0, 23)], 0.0)
    s = np.arange(512)[:, None]
    o2 = np.arange(256)[None, :]
    t2 = 6 + 2 * o2 - s
    D = np.where((t2 >= 0) & (t2 < 12), fd[np.clip(t2, 0, 11)], 0.0)
    return A, D


def _pack_consts(up_filter, down_filter):
    A, D = _filter_matrices(up_filter, down_filter)
    cm = np.zeros((128, CM_COLS), dtype=np.float16)
    cm[:, CM_A:CM_A + 512] = A.astype(np.float16)
    cm[:, CM_A2:CM_A2 + WON] = \
        (A * SQRT2)[:, WO0:WO0 + WON].astype(np.float16)
    for k, (o0, o1) in enumerate(DWIN):
        cm[:, CM_DW + 70 * k: CM_DW + 70 * k + (o1 - o0)] = \
            D[128 * k:128 * (k + 1), o0:o1].astype(np.float16)
    Dpad = np.concatenate([D, np.zeros((512, 10))], axis=1)
    for j, (k, g) in enumerate(S4MM):
        c0 = MARGIN + 128 * g
        r0 = WO0 + MC[k]
        blk = Dpad[r0:r0 + 128, c0:c0 + 128].copy()
        if k == 2:
            # rows doubled into the overlapping 4th wo-chunk
            blk[99:128, :] = 0.0
        cm[:, CM_DH + 128 * j: CM_DH + 128 * (j + 1)] = blk.astype(np.float16)
    return cm


def _run(x, bias, up_filter, down_filter, trace=False):
    import os
    # the NEFF compile cache is keyed on the HLO wrapper, which does not
    # include this kernel's BIR (it rides in backend_config) — stale-NEFF
    # collisions are possible, so always recompile
    os.environ["NEURON_FORCE_RECOMPILE"] = "1"
    from concourse.bass_utils import run_bass_kernel_spmd

    if "nc" not in _cache:
        _cache["nc"] = _build_nc()
    nc = _cache["nc"]

    cm = _pack_consts(up_filter, down_filter)
    xb = np.asarray(x, dtype=np.float32) + \
        np.asarray(bias, dtype=np.float32)[None, :, None, None]
    xf = np.ascontiguousarray(xb.astype(np.float16)
                              .reshape(NCORES * NIMG, H, W))

    in_maps = []
    for c in range(NCORES):
        in_maps.append({
            "x": xf[NIMG * c: NIMG * (c + 1)],
            "cm": cm,
            "nonce": np.zeros((1, VERSION), dtype=np.float16),
        })
    res = run_bass_kernel_spmd(nc, in_maps, core_ids=list(range(NCORES)),
                               trace=trace)
    out = np.concatenate([res.results[c]["out"][None] for c in range(NCORES)], 0)
    out = out.reshape(NCORES * NIMG, 2, 128, OUT)
    out = np.concatenate([out[:, 0, :, :], out[:, 1, 0:OUT - 128, :]], axis=1)
    out = out.reshape(4, 256, OUT, OUT)
    # device produced [wd, hd]; reference wants [hd, wd]
    out = np.ascontiguousarray(out.swapaxes(2, 3)).astype(np.float32)
    return out, res


def kernel(x, bias, up_filter, down_filter):
    out, _ = _run(x, bias, up_filter, down_filter, trace=False)
    return out


def kernel_traced(x, bias, up_filter, down_filter):
    return _run(x, bias, up_filter, down_filter, trace=True)


# revision 9
# speedup vs baseline: 1.0249x; 1.0249x over previous
"""AliasFreeActivation Trainium2 kernel (v22).

out = crop10(down2(leaky_relu(up4(x + bias)) * sqrt2))   [4,256,236,236]

Per (batch,channel) image (1024 images, 128 per core), with A the up4
matrix [128,512] and D the down2 matrix [512,256] (both banded):

  s1  v1[w,ho]  = sum_h xb[h,w] A[h,ho]              1 MM N=512
  s2  Y[ho,j]   = sum_w v1[w,ho] (sqrt2*A)[w,15+j]   4 MM N=483
      (fine-grid cols wo outside [15,498) never reach the cropped
       output, so they are never produced: N=483 not 512)
  act L = prelu(Y, 0.2)   one fused ScalarE pass per PSUM pair
          (= sqrt2*leaky_relu(up4(xb)); sqrt2 folded into s2's matrix;
           ACT Prelu honors alpha — Lrelu does NOT, its slope is 0.01)
  s3  z[wo,hd]  = sum_ho L[ho,wo] D[ho,hd]          16 MM banded N<=70
      (cropped-wo m-chunks {0,128,256,355}; the last chunk overlaps the
       third by 29 rows so every MM keeps a full-128-partition output —
       the doubled rows are zeroed in the s4 D-blocks)
  s4  oT[wd,hd] = sum_wo D[wo,wd] z[wo,hd]           5 MM N=236
      (constant D stationary -> no per-image weight loads scale with the
       data; output lands transposed and is fixed on the host)

ScalarE is the wall: every fine-grid element must cross PSUM->SBUF
through ACT or DVE at 1 elem/cycle/lane (trn2 matmul output is fp32-only,
so no 16-bit 2x DVE modes), and the trace shows ACT 100% back-to-back.
Budget per image: ACT = prelu 2x(966+~240)/1.2 + 162-col share of the
second output group's copy; DVE = V1/z/out casts + the other 74 cols;
both land ~2.34us.  Startup: the prelu table set is preloaded via a
dummy activation on a memset scratch during the const DMA, and the
A/A2 half of the const matrix is DMA'd separately so s1/s2 unblock
before the D-blocks arrive.  I/O is fp16 both ways, one input and one
output DMA per image.  s4 accumulates both 128-row output groups in ONE
PSUM bank (start=True only on the bank's first MM).

Partial-partition matmul outputs (M<128) accumulate incorrectly on this
stack — all matmuls here write full-128-partition outputs.
"""
import numpy as np

UP, DOWN, MARGIN, NEG_SLOPE = 4, 2, 10, 0.2
SQRT2 = 1.4142135623730951
H = W = 128
OUT = 236
NCORES = 8
NIMG = 128
WO0, WON = 15, 483    # fine-grid wo crop: orig wo = cropped + 15
MC = [0, 128, 256, 355]          # cropped-wo m-chunk starts (s3 lhsT)
OFSPLIT = 95                     # cols of OF group-1 evacuated on ACT

# down-matrix window per 128-row K-chunk: D[s,o] nonzero for o in [64k-3, 64k+66]
DWIN = [(0, 67), (61, 131), (125, 195), (189, 256)]
# s4 blocks (chunk k, out-group g) with g0 = wd 10..137, g1 = wd 138..265
S4MM = [(0, 0), (1, 0), (2, 0), (2, 1), (3, 1)]

CM_A = 0
CM_A2 = 512           # 483 cols (sqrt2*A, wo-cropped)
CM_DW = 996           # 4 windows, 70 cols apart (4-elem aligned)
CM_DH = 1280          # 5 dense [128,128] blocks (4-elem aligned)
CM_COLS = 1920
CM_SPLIT = 996        # DMA chunk 1 = A|A2 (unblocks s1/s2 early)
VERSION = 79          # bump on every kernel change: cache-key nonce

_cache = {}


def _build_nc(nimg=NIMG):
    import concourse.bacc as bacc
    import concourse.bass as bass
    import concourse.tile as tile
    from concourse import mybir

    F32 = mybir.dt.float32
    F16 = mybir.dt.float16
    AF = mybir.ActivationFunctionType
    ALU = mybir.AluOpType

    nc = bacc.Bacc("TRN2", target_bir_lowering=False)
    x_d = nc.dram_tensor("x", [nimg, H, W], F16, kind="ExternalInput")
    c_d = nc.dram_tensor("cm", [128, CM_COLS], F16, kind="ExternalInput")
    nc.dram_tensor("nonce", [1, VERSION], F16, kind="ExternalInput")
    o_d = nc.dram_tensor("out", [nimg, 2, 128, OUT], F16, kind="ExternalOutput")

    with tile.TileContext(nc) as tc:
        with (
            tc.tile_pool(name="const", bufs=1) as const,
            tc.tile_pool(name="xin", bufs=10) as xin,
            tc.tile_pool(name="v1p", bufs=3) as v1p,
            tc.tile_pool(name="yp", bufs=3) as yp,
            tc.tile_pool(name="zp", bufs=4) as zp,
            tc.tile_pool(name="ofp", bufs=6) as ofp,
            tc.tile_pool(name="p1", bufs=1, space="PSUM") as p1p,
            tc.tile_pool(name="p2", bufs=2, space="PSUM") as p2p,
            tc.tile_pool(name="p34", bufs=1, space="PSUM") as p34p,
            tc.tile_pool(name="pt", bufs=1, space="PSUM") as ptp,
        ):
            # warm the ACT table set (prelu) on a memset scratch so the
            # ~2.7us ACT_TABLE_LOAD overlaps the const DMA
            warm = const.tile([1, 2], F16)
            nc.vector.memset(warm[0:1, 0:1], 0.0)
            nc.scalar.activation(out=warm[0:1, 1:2], in_=warm[0:1, 0:1],
                                 func=AF.Prelu, bias=0.0, scale=1.0,
                                 alpha=NEG_SLOPE)

            cm = const.tile([128, CM_COLS], F16)
            nc.sync.dma_start(out=cm[:, 0:CM_SPLIT], in_=c_d[:, 0:CM_SPLIT])
            nc.sync.dma_start(out=cm[:, CM_SPLIT:CM_COLS],
                              in_=c_d[:, CM_SPLIT:CM_COLS])
            A_sb = cm[:, CM_A:CM_A + 512]
            A2_sb = cm[:, CM_A2:CM_A2 + WON]

            def D_sb(k):
                o0, o1 = DWIN[k]
                return cm[:, CM_DW + 70 * k: CM_DW + 70 * k + (o1 - o0)]

            def Dh_sb(j):
                return cm[:, CM_DH + 128 * j: CM_DH + 128 * (j + 1)]

            # warm PE's clock on the const DMA lane
            pwarm = p2p.tile([128, 2, 512], F32, name="p2")
            nc.tensor.matmul(out=pwarm[:32, 0, :256], lhsT=cm[:, :32],
                             rhs=cm[:, :256], start=True, stop=True)

            def s1_mm(i):
                # s1: up vertical (bias folds in during evacuation)
                X = xin.tile([128, W], F16)
                nc.sync.dma_start(out=X, in_=x_d[i])
                P1 = p1p.tile([128, 512], F32)
                nc.tensor.matmul(out=P1, lhsT=X, rhs=A_sb,
                                 start=True, stop=True)
                return P1

            def s1_evac(i, P1):
                V1 = v1p.tile([128, 512], F16)
                nc.vector.tensor_copy(out=V1, in_=P1)
                return V1

            def s4_mm(ip, Z):
                # s4: down horizontal with D stationary -> transposed out
                PT = ptp.tile([128, 2, OUT], F32)
                for j, (k, g) in enumerate(S4MM):
                    nc.tensor.matmul(out=PT[:, g, :], lhsT=Dh_sb(j),
                                     rhs=Z[:, k, :],
                                     start=(j == 0), stop=(j == len(S4MM) - 1))
                return (ip, PT)

            def of_evac(ip, PT):
                # deferred two iterations: deps are long done, so these
                # never block the engine FIFOs.  All-DVE: a third ACT
                # instruction per image exposes a ~300ns sequencer
                # semaphore stall (ACT NX sem processing is ~10x slower
                # than DVE's), costing more than the cols it offloads
                OF = ofp.tile([128, 2, OUT], F16)
                nc.vector.tensor_copy(out=OF[:, 0, :], in_=PT[:, 0, :])
                nc.vector.tensor_copy(out=OF[0:OUT - 128, 1, :],
                                      in_=PT[0:OUT - 128, 1, :])
                nc.sync.dma_start(
                    out=bass.AP(tensor=o_d[:].tensor,
                                offset=ip * 2 * 128 * OUT,
                                ap=[[OUT, 128], [128 * OUT, 2], [1, OUT]]),
                    in_=OF)

            def s2_act(V1):
                # s2 + fused leaky-relu evacuation (fine grid, wo-cropped)
                Y = yp.tile([128, 4, WON], F16)
                for pr in range(2):
                    P2 = p2p.tile([128, 2, 512], F32, name="p2")
                    for h in range(2):
                        m = 2 * pr + h
                        nc.tensor.matmul(out=P2[:, h, 0:WON],
                                         lhsT=V1[:, 128 * m:128 * (m + 1)],
                                         rhs=A2_sb, start=True, stop=True)
                    nc.scalar.activation(out=Y[:, 2 * pr:2 * pr + 2, :],
                                         in_=P2[:, :, 0:WON], func=AF.Prelu,
                                         bias=0.0, scale=1.0, alpha=NEG_SLOPE)
                return Y

            def s3_z(Y):
                # s3: down vertical (banded), all four wo-chunks in one
                # 2-bank PSUM tile, single evacuation
                P34 = p34p.tile([128, 4, 256], F32)
                for m in range(4):
                    for k in range(4):
                        o0, o1 = DWIN[k]
                        nc.tensor.matmul(
                            out=P34[:, m, o0:o1],
                            lhsT=Y[:, k, MC[m]:MC[m] + 128],
                            rhs=D_sb(k), start=(k == 0), stop=(k == 3))
                Z = zp.tile([128, 4, OUT], F16)
                nc.vector.tensor_copy(out=Z,
                                      in_=P34[:, :, MARGIN:MARGIN + OUT])
                return Z

            # software pipeline, 3 stages deep: iteration i emits
            #   of_evac(i-3) | s1/s2/prelu(i) | s3+Zcast(i-1) | s4(i-2)
            # so every instruction's producers ran a full image-period
            # earlier: s4(i-2) never waits on the Z cast (which finishes
            # during iteration i-1), the PT->OF reads are two periods
            # stale, and the PE queue [s1,s2,s3,s4] has no inline waits.
            # Single-buffer PSUM tiles stay legal because each tile's
            # reads are always emitted before its next writer.
            Ys, Zs, PTs = {}, {}, {}
            for i in range(nimg):
                if i >= 3:
                    of_evac(*PTs.pop(i - 3))
                V1 = s1_evac(i, s1_mm(i))
                Ys[i] = s2_act(V1)
                if i >= 1:
                    Zs[i - 1] = s3_z(Ys.pop(i - 1))
                if i >= 2:
                    PTs[i - 2] = s4_mm(i - 2, Zs.pop(i - 2))

            n = nimg
            Zs[n - 1] = s3_z(Ys.pop(n - 1))
            of_evac(*PTs.pop(n - 3))
            PTs[n - 2] = s4_mm(n - 2, Zs.pop(n - 2))
            of_evac(*PTs.pop(n - 2))
            PTs[n - 1] = s4_mm(n - 1, Zs.pop(n - 1))
            of_evac(*PTs.pop(n - 1))

    nc.finalize()
    return nc


def _filter_matrices(up_filter, down_filter):
    fu = np.asarray(up_filter, dtype=np.float64)
    fd = np.asarray(down_filter, dtype=np.float64)
    i = np.arange(128)[:, None]
    o = np.arange(512)[None, :]
    t = 10 + o - 4 * i
    A = np.where((t >= 0) & (t < 24), fu[np.clip(t, 0, 23)], 0.0)
    s = np.arange(512)[:, None]
    o2 = np.arange(256)[None, :]
    t2 = 6 + 2 * o2 - s
    D = np.where((t2 >= 0) & (t2 < 12), fd[np.clip(t2, 0, 11)], 0.0)
    return A, D


def _pack_consts(up_filter, down_filter):
    A, D = _filter_matrices(up_filter, down_filter)
    cm = np.zeros((128, CM_COLS), dtype=np.float16)
    cm[:, CM_A:CM_A + 512] = A.astype(np.float16)
    cm[:, CM_A2:CM_A2 + WON] = \
        (A * SQRT2)[:, WO0:WO0 + WON].astype(np.float16)
    for k, (o0, o1) in enumerate(DWIN):
        cm[:, CM_DW + 70 * k: CM_DW + 70 * k + (o1 - o0)] = \
            D[128 * k:128 * (k + 1), o0:o1].astype(np.float16)
    Dpad = np.concatenate([D, np.zeros((512, 10))], axis=1)
    for j, (k, g) in enumerate(S4MM):
        c0 = MARGIN + 128 * g
        r0 = WO0 + MC[k]
        blk = Dpad[r0:r0 + 128, c0:c0 + 128].copy()
        if k == 2:
            # rows doubled into the overlapping 4th wo-chunk
            blk[99:128, :] = 0.0
        cm[:, CM_DH + 128 * j: CM_DH + 128 * (j + 1)] = blk.astype(np.float16)
    return cm


def _run(x, bias, up_filter, down_filter, trace=False):
    import os
    # the NEFF compile cache is keyed on the HLO wrapper, which does not
    # include this kernel's BIR (it rides in backend_config) — stale-NEFF
    # collisions are possible, so always recompile
    os.environ["NEURON_FORCE_RECOMPILE"] = "1"
    from concourse.bass_utils import run_bass_kernel_spmd

    if "nc" not in _cache:
        _cache["nc"] = _build_nc()
    nc = _cache["nc"]

    cm = _pack_consts(up_filter, down_filter)
    xb = np.asarray(x, dtype=np.float32) + \
        np.asarray(bias, dtype=np.float32)[None, :, None, None]
    xf = np.ascontiguousarray(xb.astype(np.float16)
                              .reshape(NCORES * NIMG, H, W))

    in_maps = []
    for c in range(NCORES):
        in_maps.append({
            "x": xf[NIMG * c: NIMG * (c + 1)],
            "cm": cm,
            "nonce": np.zeros((1, VERSION), dtype=np.float16),
        })
    res = run_bass_kernel_spmd(nc, in_maps, core_ids=list(range(NCORES)),
                               trace=trace)
    out = np.concatenate([res.results[c]["out"][None] for c in range(NCORES)], 0)
    out = out.reshape(NCORES * NIMG, 2, 128, OUT)
    out = np.concatenate([out[:, 0, :, :], out[:, 1, 0:OUT - 128, :]], axis=1)
    out = out.reshape(4, 256, OUT, OUT)
    # device produced [wd, hd]; reference wants [hd, wd]
    out = np.ascontiguousarray(out.swapaxes(2, 3)).astype(np.float32)
    return out, res


def kernel(x, bias, up_filter, down_filter):
    out, _ = _run(x, bias, up_filter, down_filter, trace=False)
    return out


def kernel_traced(x, bias, up_filter, down_filter):
    return _run(x, bias, up_filter, down_filter, trace=True)


# revision 11
# speedup vs baseline: 1.1489x; 1.1210x over previous
"""AliasFreeActivation Trainium2 kernel (v22).

out = crop10(down2(leaky_relu(up4(x + bias)) * sqrt2))   [4,256,236,236]

Per (batch,channel) image (1024 images, 128 per core), with A the up4
matrix [128,512] and D the down2 matrix [512,256] (both banded):

  s1  v1[w,ho]  = sum_h xb[h,w] A[h,ho]              1 MM N=512
  s2  Y[ho,j]   = sum_w v1[w,ho] (sqrt2*A)[w,15+j]   4 MM N=483
      (fine-grid cols wo outside [15,498) never reach the cropped
       output, so they are never produced: N=483 not 512)
  act L = prelu(Y, 0.2)   one fused ScalarE pass per PSUM pair
          (= sqrt2*leaky_relu(up4(xb)); sqrt2 folded into s2's matrix;
           ACT Prelu honors alpha — Lrelu does NOT, its slope is 0.01)
  s3  z[wo,hd]  = sum_ho L[ho,wo] D[ho,hd]          16 MM banded N<=70
      (cropped-wo m-chunks {0,128,256,355}; the last chunk overlaps the
       third by 29 rows so every MM keeps a full-128-partition output —
       the doubled rows are zeroed in the s4 D-blocks)
  s4  oT[wd,hd] = sum_wo D[wo,wd] z[wo,hd]           5 MM N=236
      (constant D stationary -> no per-image weight loads scale with the
       data; output lands transposed and is fixed on the host)

ScalarE is the wall: every fine-grid element must cross PSUM->SBUF
through ACT or DVE at 1 elem/cycle/lane (trn2 matmul output is fp32-only,
so no 16-bit 2x DVE modes), and the trace shows ACT 100% back-to-back.
Budget per image: ACT = prelu 2x(966+~240)/1.2 + 162-col share of the
second output group's copy; DVE = V1/z/out casts + the other 74 cols;
both land ~2.34us.  Startup: the prelu table set is preloaded via a
dummy activation on a memset scratch during the const DMA, and the
A/A2 half of the const matrix is DMA'd separately so s1/s2 unblock
before the D-blocks arrive.  I/O is fp16 both ways, one input and one
output DMA per image.  s4 accumulates both 128-row output groups in ONE
PSUM bank (start=True only on the bank's first MM).

Partial-partition matmul outputs (M<128) accumulate incorrectly on this
stack — all matmuls here write full-128-partition outputs.
"""
import numpy as np

UP, DOWN, MARGIN, NEG_SLOPE = 4, 2, 10, 0.2
SQRT2 = 1.4142135623730951
H = W = 128
OUT = 236
NCORES = 8
NIMG = 128
WO0, WON = 15, 483    # fine-grid wo crop: orig wo = cropped + 15
MC = [0, 128, 256, 355]          # cropped-wo m-chunk starts (s3 lhsT)
OFSPLIT = 95                     # cols of OF group-1 evacuated on ACT

# down-matrix window per 128-row K-chunk: D[s,o] nonzero for o in [64k-3, 64k+66]
DWIN = [(0, 67), (61, 131), (125, 195), (189, 256)]
# s4 blocks (chunk k, out-group g) with g0 = wd 10..137, g1 = wd 138..265
S4MM = [(0, 0), (1, 0), (2, 0), (2, 1), (3, 1)]

CM_A = 0
CM_A2 = 512           # 483 cols (sqrt2*A, wo-cropped)
CM_DW = 996           # 4 windows, 70 cols apart (4-elem aligned)
CM_DH = 1280          # 5 dense [128,128] blocks (4-elem aligned)
CM_COLS = 1920
CM_SPLIT = 996        # DMA chunk 1 = A|A2 (unblocks s1/s2 early)
VERSION = 79          # bump on every kernel change: cache-key nonce

_cache = {}


def _build_nc(nimg=NIMG):
    import concourse.bacc as bacc
    import concourse.bass as bass
    import concourse.tile as tile
    from concourse import mybir

    F32 = mybir.dt.float32
    F16 = mybir.dt.float16
    AF = mybir.ActivationFunctionType
    ALU = mybir.AluOpType

    nc = bacc.Bacc("TRN2", target_bir_lowering=False)
    x_d = nc.dram_tensor("x", [nimg, H, W], F16, kind="ExternalInput")
    c_d = nc.dram_tensor("cm", [128, CM_COLS], F16, kind="ExternalInput")
    nc.dram_tensor("nonce", [1, VERSION], F16, kind="ExternalInput")
    o_d = nc.dram_tensor("out", [nimg, 2, 128, OUT], F16, kind="ExternalOutput")

    with tile.TileContext(nc) as tc:
        with (
            tc.tile_pool(name="const", bufs=1) as const,
            tc.tile_pool(name="xin", bufs=10) as xin,
            tc.tile_pool(name="v1p", bufs=4) as v1p,
            tc.tile_pool(name="yp", bufs=4) as yp,
            tc.tile_pool(name="zp", bufs=4) as zp,
            tc.tile_pool(name="ofp", bufs=8) as ofp,
            tc.tile_pool(name="p1", bufs=1, space="PSUM") as p1p,
            tc.tile_pool(name="p2", bufs=2, space="PSUM") as p2p,
            tc.tile_pool(name="p34", bufs=1, space="PSUM") as p34p,
            tc.tile_pool(name="pt", bufs=1, space="PSUM") as ptp,
        ):
            # warm the ACT table set (prelu) on a memset scratch so the
            # ~2.7us ACT_TABLE_LOAD overlaps the const DMA
            warm = const.tile([1, 2], F16)
            nc.vector.memset(warm[0:1, 0:1], 0.0)
            nc.scalar.activation(out=warm[0:1, 1:2], in_=warm[0:1, 0:1],
                                 func=AF.Prelu, bias=0.0, scale=1.0,
                                 alpha=NEG_SLOPE)

            cm = const.tile([128, CM_COLS], F16)
            nc.sync.dma_start(out=cm[:, 0:CM_SPLIT], in_=c_d[:, 0:CM_SPLIT])
            nc.sync.dma_start(out=cm[:, CM_SPLIT:CM_COLS],
                              in_=c_d[:, CM_SPLIT:CM_COLS])
            A_sb = cm[:, CM_A:CM_A + 512]
            A2_sb = cm[:, CM_A2:CM_A2 + WON]

            def D_sb(k):
                o0, o1 = DWIN[k]
                return cm[:, CM_DW + 70 * k: CM_DW + 70 * k + (o1 - o0)]

            def Dh_sb(j):
                return cm[:, CM_DH + 128 * j: CM_DH + 128 * (j + 1)]

            # warm PE's clock on the const DMA lane
            pwarm = p2p.tile([128, 2, 512], F32, name="p2")
            nc.tensor.matmul(out=pwarm[:32, 0, :256], lhsT=cm[:, :32],
                             rhs=cm[:, :256], start=True, stop=True)

            def s1_mm(i):
                # s1: up vertical (bias folds in during evacuation)
                X = xin.tile([128, W], F16)
                nc.sync.dma_start(out=X, in_=x_d[i])
                P1 = p1p.tile([128, 512], F32)
                nc.tensor.matmul(out=P1, lhsT=X, rhs=A_sb,
                                 start=True, stop=True)
                return P1

            def s1_evac(i, P1):
                V1 = v1p.tile([128, 512], F16)
                nc.vector.tensor_copy(out=V1, in_=P1)
                return V1

            def s4_mm(ip, Z):
                # s4: down horizontal with D stationary -> transposed out
                PT = ptp.tile([128, 2, OUT], F32)
                for j, (k, g) in enumerate(S4MM):
                    nc.tensor.matmul(out=PT[:, g, :], lhsT=Dh_sb(j),
                                     rhs=Z[:, k, :],
                                     start=(j == 0), stop=(j == len(S4MM) - 1))
                return (ip, PT)

            def of_evac(ip, PT):
                # deferred two iterations: deps are long done, so these
                # never block the engine FIFOs.  All-DVE: a third ACT
                # instruction per image exposes a ~300ns sequencer
                # semaphore stall (ACT NX sem processing is ~10x slower
                # than DVE's), costing more than the cols it offloads
                OF = ofp.tile([128, 2, OUT], F16)
                # one copy for both output groups: g1 rows 108..127 are
                # valid zeros in PT (zero-padded D columns) and the host
                # slices them off, so copying them beats a second
                # instruction's init bubble
                nc.vector.tensor_copy(out=OF, in_=PT)
                nc.sync.dma_start(
                    out=bass.AP(tensor=o_d[:].tensor,
                                offset=ip * 2 * 128 * OUT,
                                ap=[[OUT, 128], [128 * OUT, 2], [1, OUT]]),
                    in_=OF)

            def s2_act(V1):
                # s2 + fused leaky-relu evacuation (fine grid, wo-cropped)
                Y = yp.tile([128, 4, WON], F16)
                for pr in range(2):
                    P2 = p2p.tile([128, 2, 512], F32, name="p2")
                    for h in range(2):
                        m = 2 * pr + h
                        nc.tensor.matmul(out=P2[:, h, 0:WON],
                                         lhsT=V1[:, 128 * m:128 * (m + 1)],
                                         rhs=A2_sb, start=True, stop=True)
                    nc.scalar.activation(out=Y[:, 2 * pr:2 * pr + 2, :],
                                         in_=P2[:, :, 0:WON], func=AF.Prelu,
                                         bias=0.0, scale=1.0, alpha=NEG_SLOPE)
                return Y

            def s3_z(Y):
                # s3: down vertical (banded), all four wo-chunks in one
                # 2-bank PSUM tile, single evacuation
                P34 = p34p.tile([128, 4, 256], F32)
                for m in range(4):
                    for k in range(4):
                        o0, o1 = DWIN[k]
                        nc.tensor.matmul(
                            out=P34[:, m, o0:o1],
                            lhsT=Y[:, k, MC[m]:MC[m] + 128],
                            rhs=D_sb(k), start=(k == 0), stop=(k == 3))
                Z = zp.tile([128, 4, OUT], F16)
                nc.vector.tensor_copy(out=Z,
                                      in_=P34[:, :, MARGIN:MARGIN + OUT])
                return Z

            # software pipeline, 3 stages deep: iteration i emits
            #   of_evac(i-3) | s1/s2/prelu(i) | s3+Zcast(i-1) | s4(i-2)
            # so every instruction's producers ran a full image-period
            # earlier: s4(i-2) never waits on the Z cast (which finishes
            # during iteration i-1), the PT->OF reads are two periods
            # stale, and the PE queue [s1,s2,s3,s4] has no inline waits.
            # Single-buffer PSUM tiles stay legal because each tile's
            # reads are always emitted before its next writer.
            Ys, Zs, PTs = {}, {}, {}
            for i in range(nimg):
                if i >= 3:
                    of_evac(*PTs.pop(i - 3))
                V1 = s1_evac(i, s1_mm(i))
                Ys[i] = s2_act(V1)
                if i >= 1:
                    Zs[i - 1] = s3_z(Ys.pop(i - 1))
                if i >= 2:
                    PTs[i - 2] = s4_mm(i - 2, Zs.pop(i - 2))

            n = nimg
            Zs[n - 1] = s3_z(Ys.pop(n - 1))
            of_evac(*PTs.pop(n - 3))
            PTs[n - 2] = s4_mm(n - 2, Zs.pop(n - 2))
            of_evac(*PTs.pop(n - 2))
            PTs[n - 1] = s4_mm(n - 1, Zs.pop(n - 1))
            of_evac(*PTs.pop(n - 1))

    nc.finalize()
    return nc


def _filter_matrices(up_filter, down_filter):
    fu = np.asarray(up_filter, dtype=np.float64)
    fd = np.asarray(down_filter, dtype=np.float64)
    i = np.arange(128)[:, None]
    o = np.arange(512)[None, :]
    t = 10 + o - 4 * i
    A = np.where((t >= 0) & (t < 24), fu[np.clip(t, 0, 23)], 0.0)
    s = np.arange(512)[:, None]
    o2 = np.arange(256)[None, :]
    t2 = 6 + 2 * o2 - s
    D = np.where((t2 >= 0) & (t2 < 12), fd[np.clip(t2, 0, 11)], 0.0)
    return A, D


def _pack_consts(up_filter, down_filter):
    A, D = _filter_matrices(up_filter, down_filter)
    cm = np.zeros((128, CM_COLS), dtype=np.float16)
    cm[:, CM_A:CM_A + 512] = A.astype(np.float16)
    cm[:, CM_A2:CM_A2 + WON] = \
        (A * SQRT2)[:, WO0:WO0 + WON].astype(np.float16)
    for k, (o0, o1) in enumerate(DWIN):
        cm[:, CM_DW + 70 * k: CM_DW + 70 * k + (o1 - o0)] = \
            D[128 * k:128 * (k + 1), o0:o1].astype(np.float16)
    Dpad = np.concatenate([D, np.zeros((512, 10))], axis=1)
    for j, (k, g) in enumerate(S4MM):
        c0 = MARGIN + 128 * g
        r0 = WO0 + MC[k]
        blk = Dpad[r0:r0 + 128, c0:c0 + 128].copy()
        if k == 2:
            # rows doubled into the overlapping 4th wo-chunk
            blk[99:128, :] = 0.0
        cm[:, CM_DH + 128 * j: CM_DH + 128 * (j + 1)] = blk.astype(np.float16)
    return cm


def _run(x, bias, up_filter, down_filter, trace=False):
    import os
    # the NEFF compile cache is keyed on the HLO wrapper, which does not
    # include this kernel's BIR (it rides in backend_config) — stale-NEFF
    # collisions are possible, so always recompile
    os.environ["NEURON_FORCE_RECOMPILE"] = "1"
    from concourse.bass_utils import run_bass_kernel_spmd

    if "nc" not in _cache:
        _cache["nc"] = _build_nc()
    nc = _cache["nc"]

    cm = _pack_consts(up_filter, down_filter)
    xb = np.asarray(x, dtype=np.float32) + \
        np.asarray(bias, dtype=np.float32)[None, :, None, None]
    xf = np.ascontiguousarray(xb.astype(np.float16)
                              .reshape(NCORES * NIMG, H, W))

    in_maps = []
    for c in range(NCORES):
        in_maps.append({
            "x": xf[NIMG * c: NIMG * (c + 1)],
            "cm": cm,
            "nonce": np.zeros((1, VERSION), dtype=np.float16),
        })
    res = run_bass_kernel_spmd(nc, in_maps, core_ids=list(range(NCORES)),
                               trace=trace)
    out = np.concatenate([res.results[c]["out"][None] for c in range(NCORES)], 0)
    out = out.reshape(NCORES * NIMG, 2, 128, OUT)
    out = np.concatenate([out[:, 0, :, :], out[:, 1, 0:OUT - 128, :]], axis=1)
    out = out.reshape(4, 256, OUT, OUT)
    # device produced [wd, hd]; reference wants [hd, wd]
    out = np.ascontiguousarray(out.swapaxes(2, 3)).astype(np.float32)
    return out, res


def kernel(x, bias, up_filter, down_filter):
    out, _ = _run(x, bias, up_filter, down_filter, trace=False)
    return out


def kernel_traced(x, bias, up_filter, down_filter):
    return _run(x, bias, up_filter, down_filter, trace=True)
